# revision 1
# baseline (speedup 1.0000x reference)
"""Trainium2 Bass kernel for nn_DriftScene_88270167868070.

Contract: kernel(**inputs) takes FULL unsharded inputs (as produced by
setup_inputs()) and returns the FULL output (a scalar np.float32).

Strategy (8 NeuronCores, one SPMD launch):
  - Data-parallel transformer generator over the batch (64 scenes/core),
    fp32 matmuls (precision required: the final loss is dominated by fp32
    rounding of xf + V, so xf must be fp32-accurate; bf16/tf32 generators
    fail by 5-7e-2 relative).
  - Activations resident in transposed layout X_T [d_model on partitions,
    tokens on free]; weights pre-transposed on host.
  - Matching stage (cdist + double softmax + V) row-sharded, bf16 matmuls;
    one packed AllGather shares xf (both layouts) + ||xf||^2, one AllReduce
    shares column-softmax sums.
  - loss = mean((xf - fl32(xf + V))^2) with explicit fp32 rounding.
"""

import numpy as np
from contextlib import ExitStack

import concourse.bass as bass
import concourse.tile as tile
from concourse import bacc, mybir
from concourse.bass_utils import run_bass_kernel_spmd
from concourse.masks import make_identity
import ml_dtypes

F32 = mybir.dt.float32
BF16 = mybir.dt.bfloat16
AF = mybir.ActivationFunctionType
ALU = mybir.AluOpType
AX = mybir.AxisListType

# Problem dims (hardcoded per contract)
B, L, CH = 512, 32, 128
D, HEADS, DEPTH, FF = 512, 8, 4, 2048
DH = D // HEADS
LN_EPS = 1e-5
NC_ = 8                 # cores
SC = B // NC_           # 64 scenes per core
T = SC * L              # 2048 tokens per core
TB = 256                # tokens per t-block
NB = T // TB            # 8 t-blocks
NS = TB // 128          # 2 subtiles per block
KD = D // 128           # 4 d-tiles
KF = FF // 128          # 16 ff-tiles
FDIM = L * CH           # 4096 flattened feature dim
KFl = FDIM // 128       # 32 f-tiles
M_SHIFT = -20.0         # global shift for column softmax stabilization

# packed AllGather layout (bf16 element offsets)
AG_XFT = 0                      # xf_T   [4096, 64]
AG_XFN = FDIM * SC              # xf_nat [64, 4096]
AG_XN = 2 * FDIM * SC           # xn bits: f32 [64,1] viewed as bf16 [64,2]
AG_SZ = 2 * FDIM * SC + 2 * SC  # 524416


def _build_nc():
    nc = bacc.Bacc("TRN2", target_bir_lowering=False, debug=False, num_devices=NC_)

    # ---------------- I/O ----------------
    def inp(name, shape, dt=F32):
        return nc.dram_tensor(name, shape, dt, kind="ExternalInput").ap()

    epsT = inp("epsT", [128, T])              # eps shard, [ch, tok]
    inwT = inp("inwT", [128, D])              # in_w.T
    inb = inp("inb", [D])
    wqkvT = inp("wqkvT", [DEPTH, D, 3 * D])   # Wqkv[i].T
    bqkv = inp("bqkv", [DEPTH, 3 * D])
    woT = inp("woT", [DEPTH, D, D])
    bo = inp("bo", [DEPTH, D])
    ln1g = inp("ln1g", [DEPTH, D])
    ln1b = inp("ln1b", [DEPTH, D])
    w1T = inp("w1T", [DEPTH, D, FF])
    b1 = inp("b1", [DEPTH, FF])
    w2T = inp("w2T", [DEPTH, FF, D])
    b2 = inp("b2", [DEPTH, D])
    ln2g = inp("ln2g", [DEPTH, D])
    ln2b = inp("ln2b", [DEPTH, D])
    outwT = inp("outwT", [D, CH])             # out_w.T
    outb = inp("outb", [CH])
    pT = inp("pT", [FDIM, B], BF16)           # sample_p transposed [f, scene]
    pnat = inp("pnat", [B, FDIM], BF16)       # sample_p natural
    pn_bc = inp("pn_bc", [SC, B])             # ||p_j||^2 broadcast rows
    attn_mask = inp("attn_mask", [128, 128])  # 4-scene block-diag 0/1
    negdiag = inp("negdiag", [SC, B])         # 1e6 at (i, SC*core + i)

    loss_part = nc.dram_tensor("loss_part", [1, 1], F32, kind="ExternalOutput").ap()

    # ---------------- DRAM scratch ----------------
    ag_in = nc.dram_tensor("ag_in", [AG_SZ], BF16).ap()
    ag_out = nc.dram_tensor("ag_out", [NC_ * AG_SZ], BF16, addr_space="Shared").ap()
    xf32_d = nc.dram_tensor("xf32_d", [SC, FDIM], F32).ap()
    ar_in = nc.dram_tensor("ar_in", [1, 2 * B], F32).ap()
    ar_out = nc.dram_tensor("ar_out", [1, 2 * B], F32, addr_space="Shared").ap()

    with tile.TileContext(nc) as tc, ExitStack() as ctx:
        # ---------------- pools (bufs is PER TAG) ----------------
        const = ctx.enter_context(tc.tile_pool(name="const", bufs=1))
        xTp = ctx.enter_context(tc.tile_pool(name="xT", bufs=1))
        hp = ctx.enter_context(tc.tile_pool(name="h", bufs=3))
        sqp = ctx.enter_context(tc.tile_pool(name="sq", bufs=4))
        rowp = ctx.enter_context(tc.tile_pool(name="rows", bufs=3))
        mrow = ctx.enter_context(tc.tile_pool(name="mrow", bufs=1))
        bcp = ctx.enter_context(tc.tile_pool(name="bc", bufs=3))
        mbcp = ctx.enter_context(tc.tile_pool(name="mbc", bufs=1))
        bw_p = ctx.enter_context(tc.tile_pool(name="bigw", bufs=KD))
        wo_p = ctx.enter_context(tc.tile_pool(name="wo", bufs=KD))
        w2_p = ctx.enter_context(tc.tile_pool(name="w2", bufs=16))
        colp = ctx.enter_context(tc.tile_pool(name="colp", bufs=1))
        col2p = ctx.enter_context(tc.tile_pool(name="col2p", bufs=2))
        qk_p = ctx.enter_context(tc.tile_pool(name="qk", bufs=12))
        v65_p = ctx.enter_context(tc.tile_pool(name="v65", bufs=3))
        e_p = ctx.enter_context(tc.tile_pool(name="et", bufs=2))
        onat_p = ctx.enter_context(tc.tile_pool(name="onat", bufs=3))
        oT_p = ctx.enter_context(tc.tile_pool(name="oT", bufs=4))
        relu_p = ctx.enter_context(tc.tile_pool(name="relu", bufs=4))
        outp = ctx.enter_context(tc.tile_pool(name="outp", bufs=1))
        mtch = ctx.enter_context(tc.tile_pool(name="mtch", bufs=1))
        m2p = ctx.enter_context(tc.tile_pool(name="m2p", bufs=2))
        mov_p = ctx.enter_context(tc.tile_pool(name="mov", bufs=4))
        wT_p = ctx.enter_context(tc.tile_pool(name="wTp", bufs=8))

        ps_mm = ctx.enter_context(tc.tile_pool(name="ps_mm", bufs=2, space="PSUM"))
        ps_acc = ctx.enter_context(tc.tile_pool(name="ps_acc", bufs=4, space="PSUM"))
        ps_at = ctx.enter_context(tc.tile_pool(name="ps_at", bufs=2, space="PSUM"))

        # ---------------- constants ----------------
        ident = const.tile([128, 128], F32)
        make_identity(nc, ident[:])
        ones_col = const.tile([128, 1], F32)
        nc.vector.memset(ones_col[:], 1.0)
        mask_t = const.tile([128, 128], F32)
        nc.sync.dma_start(mask_t[:], attn_mask)
        pn_t = const.tile([SC, B], F32)
        nc.sync.dma_start(pn_t[:], pn_bc)
        nd_t = const.tile([SC, B], F32)
        nc.sync.dma_start(nd_t[:], negdiag)
        eps_col = const.tile([1, 1], F32)
        nc.vector.memset(eps_col[:], LN_EPS)
        m20_col = const.tile([SC, 1], F32)
        nc.vector.memset(m20_col[:], -M_SHIFT)

        # residual stream X_T: KD tiles [128, T] fp32, persistent
        xT = [xTp.tile([128, T], F32, tag=f"xT{k}", name=f"xT{k}") for k in range(KD)]

        def col(ap_1d, base, tag, pool=colp, n=128):
            t = pool.tile([n, 1], F32, tag=tag, name=tag)
            nc.sync.dma_start(t[:], ap_1d[base:base + n])
            return t

        # ========= input projection: X_T = (eps @ in_w.T).T =========
        inw_sb = bw_p.tile([128, FF], F32, tag="bigw", name="inw")
        nc.sync.dma_start(inw_sb[:, 0:D], inwT)
        ib_cols = [col(inb, k * 128, f"bo{k}") for k in range(KD)]
        for b_ in range(NB):
            bsl = slice(b_ * TB, (b_ + 1) * TB)
            eps_blk = sqp.tile([128, TB], F32, tag="sq", name="epsblk")
            nc.sync.dma_start(eps_blk[:], epsT[:, bsl])
            for dt_ in range(KD):
                ps = ps_mm.tile([128, TB], F32, tag="mm", name="ps")
                nc.tensor.matmul(ps[:], inw_sb[:, dt_ * 128:(dt_ + 1) * 128],
                                 eps_blk[:], start=True, stop=True)
                nc.scalar.activation(xT[dt_][:, bsl], ps[:],
                                     AF.Identity, bias=ib_cols[dt_][:], scale=1.0)

        # ========= per-block layernorm =========
        def ln_block(b_, g_cols, b_cols, htag):
            """LN over partition dim for tokens of block b_ -> h tiles."""
            bsl = slice(b_ * TB, (b_ + 1) * TB)
            s_row = rowp.tile([1, TB], F32, tag="srow", name="srow")
            ps_s = ps_mm.tile([1, TB], F32, tag="mm", name="ps")
            for k in range(KD):
                nc.tensor.matmul(ps_s[:], ones_col[:], xT[k][:, bsl],
                                 start=(k == 0), stop=(k == KD - 1))
            nc.vector.tensor_scalar_mul(s_row[:], ps_s[:], -1.0 / D)   # -mean
            q_row = rowp.tile([1, TB], F32, tag="qrow", name="qrow")
            ps_q = ps_mm.tile([1, TB], F32, tag="mm", name="ps")
            for k in range(KD):
                sq = sqp.tile([128, TB], F32, tag="sq", name="sq")
                nc.vector.tensor_mul(sq[:], xT[k][:, bsl], xT[k][:, bsl])
                nc.tensor.matmul(ps_q[:], ones_col[:], sq[:],
                                 start=(k == 0), stop=(k == KD - 1))
            msq = rowp.tile([1, TB], F32, tag="msq", name="msq")
            nc.vector.tensor_mul(msq[:], s_row[:], s_row[:])
            # var = q/D - m^2  (into q_row)
            nc.vector.scalar_tensor_tensor(q_row[:], ps_q[:], 1.0 / D, msq[:],
                                           op0=ALU.mult, op1=ALU.subtract)
            # rstd = 1/sqrt(var + eps): sqrt into msq, recip into q_row
            nc.scalar.activation(msq[:], q_row[:], AF.Sqrt, bias=eps_col[:], scale=1.0)
            nc.vector.reciprocal(q_row[:], msq[:])
            # shift = -m * rstd (into s_row)
            nc.vector.tensor_mul(s_row[:], s_row[:], q_row[:])
            rstd_bc = bcp.tile([128, TB], F32, tag="rstd_bc", name="rstdbc")
            nc.gpsimd.partition_broadcast(rstd_bc[:], q_row[:])
            shift_bc = bcp.tile([128, TB], F32, tag="shift_bc", name="shiftbc")
            nc.gpsimd.partition_broadcast(shift_bc[:], s_row[:])
            hs = []
            for k in range(KD):
                hh = hp.tile([128, TB], F32, tag=f"{htag}{k}", name=f"{htag}{k}")
                nc.vector.tensor_mul(hh[:], xT[k][:, bsl], rstd_bc[:])
                nc.vector.tensor_add(hh[:], hh[:], shift_bc[:])
                nc.scalar.activation(hh[:], hh[:], AF.Identity,
                                     bias=b_cols[k][:], scale=g_cols[k][:])
                hs.append(hh)
            return hs

        # ========= transformer layers =========
        for li in range(DEPTH):
            # ---- attention phase ----
            g1c = [col(ln1g[li], k * 128, f"lng{k}") for k in range(KD)]
            lb1c = [col(ln1b[li], k * 128, f"lnb{k}") for k in range(KD)]
            wq_sl = []
            for k in range(KD):
                w = bw_p.tile([128, FF], F32, tag="bigw", name="wqs")
                nc.sync.dma_start(w[:, 0:3 * D], wqkvT[li, k * 128:(k + 1) * 128, :])
                wq_sl.append(w)
            wo_sl = []
            for k in range(KD):
                w = wo_p.tile([128, D], F32, tag="wo", name="wos")
                nc.sync.dma_start(w[:], woT[li, k * 128:(k + 1) * 128, :])
                wo_sl.append(w)
            bq_cols = [col(bqkv[li], ot * 128, f"bq{ot}") for ot in range(8)]
            bv_bc = bcp.tile([128, D], F32, tag="bvbc", name="bvbc", bufs=1)
            nc.gpsimd.dma_start(bv_bc[:], bass.AP(
                tensor=bqkv.tensor, offset=bqkv.offset + li * 3 * D + 2 * D,
                ap=[[0, 128], [1, D]]))
            bo_cols = [col(bo[li], ot * 128, f"bo{ot}") for ot in range(KD)]

            for b_ in range(NB):
                tsl = slice(b_ * TB, (b_ + 1) * TB)
                h = ln_block(b_, g1c, lb1c, "h")
                # Q,K projections (transposed out)
                qk = []
                for ot in range(8):
                    ps = ps_mm.tile([128, TB], F32, tag="mm", name="ps")
                    for k in range(KD):
                        nc.tensor.matmul(ps[:], wq_sl[k][:, ot * 128:(ot + 1) * 128],
                                         h[k][:], start=(k == 0), stop=(k == KD - 1))
                    t = qk_p.tile([128, TB], F32, tag="qk", name="qk")
                    nc.scalar.activation(t[:], ps[:], AF.Identity,
                                         bias=bq_cols[ot][:], scale=1.0)
                    qk.append(t)
                onats = []
                for tt in range(NS):
                    ssl = slice(tt * 128, (tt + 1) * 128)
                    # V natural for this subtile, 65-strided with ones column
                    ps = ps_mm.tile([128, D], F32, tag="mm", name="ps")
                    for k in range(KD):
                        nc.tensor.matmul(ps[:], h[k][:, ssl], wq_sl[k][:, 2 * D:3 * D],
                                         start=(k == 0), stop=(k == KD - 1))
                    v = v65_p.tile([128, 8 * 65], F32, tag="v65", name="v65")
                    nc.vector.memset(
                        v[:].rearrange("p (hh c) -> p hh c", hh=8)[:, :, 64:65], 1.0)
                    for hh in range(8):
                        nc.vector.tensor_add(v[:, hh * 65:hh * 65 + 64],
                                             ps[:, hh * 64:(hh + 1) * 64],
                                             bv_bc[:, hh * 64:(hh + 1) * 64])
                    # attention
                    onat = onat_p.tile([128, D], F32, tag="onat", name="onat")
                    for hh in range(8):
                        bp = (hh % 2) * 64
                        kt = qk[4 + hh // 2]
                        qt = qk[hh // 2]
                        s_ps = ps_at.tile([128, 128], F32, tag="at", name="sps")
                        nc.tensor.matmul(s_ps[:], kt[bp:bp + 64, ssl], qt[bp:bp + 64, ssl],
                                         start=True, stop=True)
                        et = e_p.tile([128, 128], F32, tag="et", name="et")
                        nc.scalar.activation(et[:], s_ps[:], AF.Exp, bias=0.0, scale=0.125)
                        nc.vector.tensor_mul(et[:], et[:], mask_t[:])
                        o_ps = ps_at.tile([128, 65], F32, tag="at", name="ops")
                        nc.tensor.matmul(o_ps[:], et[:], v[:, hh * 65:(hh + 1) * 65],
                                         start=True, stop=True)
                        rcol = col2p.tile([128, 1], F32, tag="rcol", name="rcol")
                        nc.vector.reciprocal(rcol[:], o_ps[:, 64:65])
                        nc.vector.tensor_scalar_mul(onat[:, hh * 64:(hh + 1) * 64],
                                                    o_ps[:, 0:64], rcol[:])
                    onats.append(onat)
                # transpose O -> O_T
                oT = [oT_p.tile([128, TB], F32, tag="oT", name="oT") for _ in range(KD)]
                for tt in range(NS):
                    for k in range(KD):
                        tp = ps_at.tile([128, 128], F32, tag="at", name="tp")
                        nc.tensor.transpose(tp[:], onats[tt][:, k * 128:(k + 1) * 128],
                                            ident[:])
                        nc.vector.tensor_copy(oT[k][:, tt * 128:(tt + 1) * 128], tp[:])
                # Wo + residual
                for ot in range(KD):
                    ps = ps_mm.tile([128, TB], F32, tag="mm", name="ps")
                    for k in range(KD):
                        nc.tensor.matmul(ps[:], wo_sl[k][:, ot * 128:(ot + 1) * 128],
                                         oT[k][:], start=(k == 0), stop=(k == KD - 1))
                    nc.vector.scalar_tensor_tensor(xT[ot][:, tsl], ps[:], bo_cols[ot][:],
                                                   xT[ot][:, tsl], op0=ALU.add, op1=ALU.add)

            # ---- FF phase ----
            g2c = [col(ln2g[li], k * 128, f"lng{k}") for k in range(KD)]
            lb2c = [col(ln2b[li], k * 128, f"lnb{k}") for k in range(KD)]
            w1_sl = []
            for k in range(KD):
                w = bw_p.tile([128, FF], F32, tag="bigw", name="w1s")
                nc.sync.dma_start(w[:], w1T[li, k * 128:(k + 1) * 128, :])
                w1_sl.append(w)
            w2_sl = []
            for kf in range(KF):
                w = w2_p.tile([128, D], F32, tag="w2", name="w2s")
                nc.sync.dma_start(w[:], w2T[li, kf * 128:(kf + 1) * 128, :])
                w2_sl.append(w)
            bff_cols = [col(b2[li], ot * 128, f"bo{ot}") for ot in range(KD)]
            for b_ in range(NB):
                tsl = slice(b_ * TB, (b_ + 1) * TB)
                h2 = ln_block(b_, g2c, lb2c, "h")
                acc = [ps_acc.tile([128, TB], F32, tag="acc", name="facc")[:]
                       for _ in range(KD)]
                for kf in range(KF):
                    ps = ps_mm.tile([128, TB], F32, tag="mm", name="ps")
                    for k in range(KD):
                        nc.tensor.matmul(ps[:], w1_sl[k][:, kf * 128:(kf + 1) * 128],
                                         h2[k][:], start=(k == 0), stop=(k == KD - 1))
                    b1col = col(b1[li], kf * 128, "b1c", pool=col2p)
                    rl = relu_p.tile([128, TB], F32, tag="relu", name="rl")
                    nc.scalar.activation(rl[:], ps[:], AF.Relu, bias=b1col[:], scale=1.0)
                    for ot in range(KD):
                        nc.tensor.matmul(acc[ot], w2_sl[kf][:, ot * 128:(ot + 1) * 128],
                                         rl[:], start=(kf == 0), stop=(kf == KF - 1))
                for ot in range(KD):
                    nc.vector.scalar_tensor_tensor(xT[ot][:, tsl], acc[ot],
                                                   bff_cols[ot][:], xT[ot][:, tsl],
                                                   op0=ALU.add, op1=ALU.add)

        # ========= output projection =========
        outw_sb = bw_p.tile([128, FF], F32, tag="bigw", name="outw")
        for k in range(KD):
            nc.sync.dma_start(outw_sb[:, k * 128:k * 128 + CH],
                              outwT[k * 128:(k + 1) * 128, :])
        outb_col = col(outb, 0, "outbcol")
        outb_bc = mbcp.tile([128, CH], F32, tag="outbbc", name="outbbc")
        nc.gpsimd.dma_start(outb_bc[:], bass.AP(
            tensor=outb.tensor, offset=outb.offset, ap=[[0, 128], [1, CH]]))

        # y_T [ch, tok] in bf16 (stationary source for matching matmuls)
        yT_bf = outp.tile([128, T], BF16, tag="yTbf")
        for b_ in range(NB):
            ps = ps_mm.tile([128, TB], F32, tag="mm", name="ps")
            for k in range(KD):
                nc.tensor.matmul(ps[:], outw_sb[:, k * 128:k * 128 + CH],
                                 xT[k][:, b_ * TB:(b_ + 1) * TB],
                                 start=(k == 0), stop=(k == KD - 1))
            nc.scalar.activation(yT_bf[:, b_ * TB:(b_ + 1) * TB], ps[:], AF.Identity,
                                 bias=outb_col[:], scale=1.0)

        # y natural -> xf32_d DRAM fp32; bf16 shards straight into ag_in
        for tt in range(T // 128):
            ps = ps_at.tile([128, CH], F32, tag="at", name="yn_ps")
            for k in range(KD):
                nc.tensor.matmul(ps[:], xT[k][:, tt * 128:(tt + 1) * 128],
                                 outw_sb[:, k * 128:k * 128 + CH],
                                 start=(k == 0), stop=(k == KD - 1))
            yn = m2p.tile([128, CH], F32, tag="yn", name="yn")
            nc.vector.tensor_add(yn[:], ps[:], outb_bc[:])
            nc.sync.dma_start(
                xf32_d[tt * 4:(tt + 1) * 4, :].rearrange("p (l c) -> p l c", l=L),
                yn[:])
            ynbf = m2p.tile([128, CH], BF16, tag="ynbf", name="ynbf")
            nc.vector.tensor_copy(ynbf[:], yn[:])
            nc.sync.dma_start(
                ag_in[AG_XFN + tt * 4 * FDIM: AG_XFN + (tt + 1) * 4 * FDIM]
                .rearrange("(i l c) -> i l c", l=L, c=CH),
                ynbf[:])

        # xn = ||xf_i||^2 via gram diag (bf16 inputs, fp32 accum)
        xfT_st = yT_bf[:].rearrange("c (i l) -> c l i", l=L)   # [128, 32, 64]
        gram = ps_at.tile([SC, SC], F32, tag="at", name="gram")
        for l in range(KFl):
            nc.tensor.matmul(gram[:], xfT_st[:, l, :], xfT_st[:, l, :],
                             start=(l == 0), stop=(l == KFl - 1))
        gd = m2p.tile([SC, SC], F32, tag="gd", name="gd")
        nc.vector.tensor_mul(gd[:], gram[:], ident[0:SC, 0:SC])
        xn_col = colp.tile([SC, 1], F32, tag="xncol", name="xncol")
        nc.vector.reduce_sum(xn_col[:], gd[:], axis=AX.X)

        # write AG input: xf_T + xn bits (xf_nat already streamed above)
        for l in range(L):
            nc.sync.dma_start(
                ag_in[AG_XFT + l * 128 * SC: AG_XFT + (l + 1) * 128 * SC]
                .rearrange("(c i) -> c i", c=128),
                xfT_st[:, l, :])
        nc.sync.dma_start(
            ag_in[AG_XN:AG_XN + 2 * SC].rearrange("(i bb) -> i bb", bb=2),
            xn_col[:].bitcast(BF16))
        nc.gpsimd.collective_compute(
            "AllGather", ALU.bypass, replica_groups=[list(range(NC_))],
            ins=[ag_in[:]], outs=[ag_out[:]])

        # xn_full row [1, 512] f32 + broadcast
        ago_f32 = ag_out.bitcast(F32)
        xn_row = mrow.tile([1, B], F32, tag="mr", name="xnrow")
        nc.sync.dma_start(
            xn_row[:],
            bass.AP(tensor=ago_f32.tensor, offset=ago_f32.offset + AG_XN // 2,
                    ap=[[1, 1], [AG_SZ // 2, NC_], [1, SC]]))
        xn_bc = mbcp.tile([SC, B], F32, tag="mbc", name="xnbc")
        nc.gpsimd.partition_broadcast(xn_bc[:], xn_row[:])

        # S_pos / S_neg (bf16 matmuls, fp32 accum)
        spos = ps_acc.tile([SC, B], F32, tag="acc", name="spos")
        for l in range(KFl):
            mv = mov_p.tile([128, B], BF16, tag="mv", name="mv")
            nc.sync.dma_start(mv[:], pT[l * 128:(l + 1) * 128, :])
            nc.tensor.matmul(spos[:], xfT_st[:, l, :], mv[:],
                             start=(l == 0), stop=(l == KFl - 1))
        sneg = ps_acc.tile([SC, B], F32, tag="acc", name="sneg")
        for l in range(KFl):
            mv = mov_p.tile([128, B], BF16, tag="mv", name="mv")
            nc.sync.dma_start(
                mv[:],
                bass.AP(tensor=ag_out.tensor, offset=ag_out.offset + AG_XFT + l * 128 * SC,
                        ap=[[SC, 128], [AG_SZ, NC_], [1, SC]]))
            nc.tensor.matmul(sneg[:], xfT_st[:, l, :], mv[:],
                             start=(l == 0), stop=(l == KFl - 1))

        # distances -> logits -> E (in place)
        dist = mtch.tile([SC, 2 * B], F32, tag="dist")
        nc.vector.scalar_tensor_tensor(dist[:, 0:B], spos[:], -2.0, pn_t[:],
                                       op0=ALU.mult, op1=ALU.add)
        nc.vector.scalar_tensor_tensor(dist[:, B:2 * B], sneg[:], -2.0, xn_bc[:],
                                       op0=ALU.mult, op1=ALU.add)
        nc.vector.tensor_scalar_add(dist[:], dist[:], xn_col[:])
        nc.vector.tensor_scalar_max(dist[:], dist[:], 0.0)
        nc.scalar.activation(dist[:], dist[:], AF.Sqrt, bias=0.0, scale=1.0)
        nc.vector.tensor_add(dist[:, B:2 * B], dist[:, B:2 * B], nd_t[:])
        dmin = colp.tile([SC, 1], F32, tag="dmin", name="dmin")
        nc.vector.tensor_reduce(out=dmin[:], in_=dist[:], axis=AX.X, op=ALU.min)
        E = dist  # in place: E = exp(-d + dmin)
        nc.scalar.activation(E[:], dist[:], AF.Exp, bias=dmin[:], scale=-1.0)
        g_col = colp.tile([SC, 1], F32, tag="gcol", name="gcol")
        nc.scalar.activation(g_col[:], dmin[:], AF.Exp, bias=m20_col[:], scale=-1.0)
        sr_col = colp.tile([SC, 1], F32, tag="srcol", name="srcol")
        nc.vector.reduce_sum(sr_col[:], E[:], axis=AX.X)
        # partial colsums of G = E * g_i via g-weighted stationary
        cs_row = mrow.tile([1, 2 * B], F32, tag="mr", name="csrow")
        for b_ in range(2):
            ps = ps_mm.tile([1, B], F32, tag="mm", name="ps")
            nc.tensor.matmul(ps[:], g_col[:], E[:, b_ * B:(b_ + 1) * B],
                             start=True, stop=True)
            nc.vector.tensor_copy(cs_row[:, b_ * B:(b_ + 1) * B], ps[:])
        nc.sync.dma_start(ar_in, cs_row[:])
        nc.gpsimd.collective_compute(
            "AllReduce", ALU.add, replica_groups=[list(range(NC_))],
            ins=[ar_in[:]], outs=[ar_out[:]])
        cs_g = mrow.tile([1, 2 * B], F32, tag="mr", name="csg")
        nc.sync.dma_start(cs_g[:], ar_out)
        cs_bc = mbcp.tile([SC, 2 * B], F32, tag="csbc", name="csbc")
        nc.gpsimd.partition_broadcast(cs_bc[:], cs_g[:])
        nc.scalar.activation(cs_bc[:], cs_bc[:], AF.Sqrt, bias=0.0, scale=1.0)
        nc.vector.reciprocal(cs_bc[:], cs_bc[:])
        # E' = E * invsqrt(Sc); row scalars BEFORE overwriting E with W
        nc.vector.tensor_mul(E[:], E[:], cs_bc[:])
        snp = colp.tile([SC, 1], F32, tag="snp", name="snp")
        nc.vector.reduce_sum(snp[:], E[:, B:2 * B], axis=AX.X)
        spp = colp.tile([SC, 1], F32, tag="spp", name="spp")
        nc.vector.reduce_sum(spp[:], E[:, 0:B], axis=AX.X)
        tcol = colp.tile([SC, 1], F32, tag="tcol", name="tcol")
        nc.vector.reciprocal(tcol[:], sr_col[:])
        nc.vector.tensor_mul(tcol[:], tcol[:], g_col[:])
        ccol = colp.tile([SC, 1], F32, tag="ccol", name="ccol")
        nc.scalar.activation(ccol[:], tcol[:], AF.Sqrt, bias=0.0, scale=1.0)
        alpha = colp.tile([SC, 1], F32, tag="alpha", name="alpha")
        nc.vector.tensor_mul(alpha[:], tcol[:], snp[:])
        beta = colp.tile([SC, 1], F32, tag="beta", name="beta")
        nc.vector.tensor_mul(beta[:], alpha[:], spp[:])
        nc.vector.tensor_mul(beta[:], beta[:], ccol[:])
        nc.vector.tensor_scalar_mul(beta[:], beta[:], -1.0)
        # W = E' * alpha / -beta (in place), transpose, cast bf16
        nc.vector.tensor_scalar_mul(E[:, 0:B], E[:, 0:B], alpha[:])
        nc.vector.tensor_scalar_mul(E[:, B:2 * B], E[:, B:2 * B], beta[:])
        wT = []
        for half in range(2):
            for jt in range(4):
                tp = ps_at.tile([128, SC], F32, tag="at", name="wtp")
                nc.tensor.transpose(
                    tp[:], E[:, half * B + jt * 128: half * B + (jt + 1) * 128],
                    ident[0:SC, 0:SC])
                t = wT_p.tile([128, SC], BF16, tag="wT", name="wT")
                nc.vector.tensor_copy(t[:], tp[:])
                wT.append(t)
        # V and loss: V = Wpos @ p - Wneg @ xf_full, r = xf - fl(xf + V)
        lacc = m2p.tile([SC, 16], F32, tag="lacc", name="lacc", bufs=1)
        FBW = 256
        for fb in range(FDIM // FBW):
            vps = ps_acc.tile([SC, FBW], F32, tag="acc", name="vps")
            for jt in range(4):
                mv = mov_p.tile([128, FBW], BF16, tag="mv", name="mv")
                nc.sync.dma_start(mv[:], pnat[jt * 128:(jt + 1) * 128,
                                              fb * FBW:(fb + 1) * FBW])
                nc.tensor.matmul(vps[:], wT[jt][:], mv[:], start=(jt == 0), stop=False)
            for jt in range(4):
                mv = mov_p.tile([128, FBW], BF16, tag="mv", name="mv")
                nc.sync.dma_start(
                    mv[:],
                    bass.AP(tensor=ag_out.tensor,
                            offset=ag_out.offset + AG_XFN + 2 * jt * AG_SZ + fb * FBW,
                            ap=[[AG_SZ, 2], [FDIM, SC], [1, FBW]]))
                nc.tensor.matmul(vps[:], wT[4 + jt][:], mv[:], start=False, stop=(jt == 3))
            xfb = m2p.tile([SC, FBW], F32, tag="xfb", name="xfb")
            nc.sync.dma_start(xfb[:], xf32_d[:, fb * FBW:(fb + 1) * FBW])
            t1 = m2p.tile([SC, FBW], F32, tag="t1", name="t1")
            nc.vector.tensor_add(t1[:], xfb[:], vps[:])
            nc.vector.tensor_sub(t1[:], xfb[:], t1[:])
            nc.vector.tensor_mul(t1[:], t1[:], t1[:])
            nc.vector.reduce_sum(lacc[:, fb:fb + 1], t1[:], axis=AX.X)
        lsum = colp.tile([SC, 1], F32, tag="lsum", name="lsum")
        nc.vector.reduce_sum(lsum[:], lacc[:], axis=AX.X)
        tot = ps_mm.tile([1, 1], F32, tag="mm", name="tot")
        nc.tensor.matmul(tot[:], ones_col[0:SC, :], lsum[:], start=True, stop=True)
        tot_sb = colp.tile([1, 1], F32, tag="tot", name="totsb")
        nc.vector.tensor_copy(tot_sb[:], tot[:])
        nc.sync.dma_start(loss_part, tot_sb[:])

    nc.compile()
    return nc


_NC_CACHE = None


def _get_nc():
    global _NC_CACHE
    if _NC_CACHE is None:
        _NC_CACHE = _build_nc()
    return _NC_CACHE


def _prep_inputs(inputs):
    f32 = lambda x: np.ascontiguousarray(np.asarray(x), dtype=np.float32)
    bf = lambda x: np.ascontiguousarray(np.asarray(x, dtype=ml_dtypes.bfloat16))
    sample_p = f32(inputs["sample_p"])
    eps = f32(inputs["eps"])
    p2 = sample_p.reshape(B, FDIM)
    pn = (p2.astype(np.float64) ** 2).sum(-1).astype(np.float32)

    common = {
        "inwT": f32(inputs["in_w"]).T.copy(),
        "inb": f32(inputs["in_b"]),
        "wqkvT": np.ascontiguousarray(f32(inputs["Wqkv"]).transpose(0, 2, 1)),
        "bqkv": f32(inputs["bqkv"]),
        "woT": np.ascontiguousarray(f32(inputs["Wo"]).transpose(0, 2, 1)),
        "bo": f32(inputs["bo"]),
        "ln1g": f32(inputs["ln1_g"]), "ln1b": f32(inputs["ln1_b"]),
        "w1T": np.ascontiguousarray(f32(inputs["W1"]).transpose(0, 2, 1)),
        "b1": f32(inputs["b1"]),
        "w2T": np.ascontiguousarray(f32(inputs["W2"]).transpose(0, 2, 1)),
        "b2": f32(inputs["b2"]),
        "ln2g": f32(inputs["ln2_g"]), "ln2b": f32(inputs["ln2_b"]),
        "outwT": f32(inputs["out_w"]).T.copy(),
        "outb": f32(inputs["out_b"]),
        "pT": bf(p2.T),
        "pnat": bf(p2),
        "pn_bc": np.broadcast_to(pn[None, :], (SC, B)).copy(),
        "attn_mask": np.kron(np.eye(4, dtype=np.float32), np.ones((32, 32), np.float32)),
    }
    in_maps = []
    for c in range(NC_):
        nd = np.zeros((SC, B), np.float32)
        nd[np.arange(SC), SC * c + np.arange(SC)] = 1e6
        m = dict(common)
        m["epsT"] = eps[c * SC:(c + 1) * SC].reshape(T, CH).T.copy()
        m["negdiag"] = nd
        in_maps.append(m)
    return in_maps


def kernel(**inputs) -> np.ndarray:
    nc = _get_nc()
    in_maps = _prep_inputs(inputs)
    res = run_bass_kernel_spmd(nc, in_maps, list(range(NC_)))
    total = sum(float(r["loss_part"][0, 0]) for r in res.results)
    return np.float32(total / (B * FDIM))



# revision 7
# speedup vs baseline: 1.5405x; 1.5405x over previous
"""Trainium2 Bass kernel for nn_DriftScene_88270167868070.

Contract: kernel(**inputs) takes FULL unsharded inputs (as produced by
setup_inputs()) and returns the FULL output (a scalar np.float32).

Strategy (8 NeuronCores, one SPMD launch):
  - Data-parallel transformer generator over the batch (64 scenes/core),
    fp32 matmuls (precision required: the loss measures which tiny V
    entries survive fp32 rounding of xf + V, so xf must be fp32-exact;
    bf16/tf32/fp16 anywhere in the generator fails at 2-5e-2 relative).
  - Matching stage row-sharded, bf16 matmuls, fully SBUF-resident:
    AllGather ships xf_nat (bf16) + ||xf||^2 only; remote xf^T tiles are
    rebuilt with PE transposes (no strided DMA); p tiles stream with
    contiguous lines; V matmuls run entirely from SBUF.
  - loss = mean((xf - fl32(xf + V))^2) with explicit fp32 rounding.
"""

import numpy as np
from contextlib import ExitStack

import concourse.bass as bass
import concourse.tile as tile
from concourse import bacc, mybir
from concourse.bass_utils import run_bass_kernel_spmd
from concourse.masks import make_identity
import ml_dtypes

F32 = mybir.dt.float32
BF16 = mybir.dt.bfloat16
AF = mybir.ActivationFunctionType
ALU = mybir.AluOpType
AX = mybir.AxisListType

# Problem dims (hardcoded per contract)
B, L, CH = 512, 32, 128
D, HEADS, DEPTH, FF = 512, 8, 4, 2048
DH = D // HEADS
LN_EPS = 1e-5
NC_ = 8                 # cores
SC = B // NC_           # 64 scenes per core
T = SC * L              # 2048 tokens per core
TB = 256                # tokens per t-block
NB = T // TB            # 8 t-blocks
NS = TB // 128          # 2 subtiles per block
KD = D // 128           # 4 d-tiles
KF = FF // 128          # 16 ff-tiles
FDIM = L * CH           # 4096 flattened feature dim
KFl = FDIM // 128       # 32 f-tiles
M_SHIFT = -20.0         # global shift for column softmax stabilization

# packed AllGather layout (bf16 element offsets): xf_nat + xn bits
AG_XFN = 0                      # xf_nat [64, 4096]
AG_XN = FDIM * SC               # xn bits: f32 [64,1] viewed as bf16 [64,2]
AG_SZ = FDIM * SC + 2 * SC      # 262272


def _build_nc():
    nc = bacc.Bacc("TRN2", target_bir_lowering=False, debug=False, num_devices=NC_)

    # ---------------- I/O ----------------
    def inp(name, shape, dt=F32):
        return nc.dram_tensor(name, shape, dt, kind="ExternalInput").ap()

    epsT = inp("epsT", [128, T])              # eps shard, [ch, tok]
    inwT = inp("inwT", [128, D])              # in_w.T
    inb = inp("inb", [D])
    wqkvT = inp("wqkvT", [DEPTH, D, 3 * D])   # Wqkv[i].T
    bqkv = inp("bqkv", [DEPTH, 3 * D])
    woT = inp("woT", [DEPTH, D, D])
    bo = inp("bo", [DEPTH, D])
    ln1g = inp("ln1g", [DEPTH, D])
    ln1b = inp("ln1b", [DEPTH, D])
    w1T = inp("w1T", [DEPTH, D, FF])
    b1 = inp("b1", [DEPTH, FF])
    w2T = inp("w2T", [DEPTH, FF, D])
    b2 = inp("b2", [DEPTH, D])
    ln2g = inp("ln2g", [DEPTH, D])
    ln2b = inp("ln2b", [DEPTH, D])
    outwT = inp("outwT", [D, CH])             # out_w.T
    outb = inp("outb", [CH])
    pT = inp("pT", [FDIM, B], BF16)           # sample_p transposed [f, scene]
    pnat = inp("pnat", [B, FDIM], BF16)       # sample_p natural
    pn_bc = inp("pn_bc", [SC, B])             # ||p_j||^2 broadcast rows
    attn_mask = inp("attn_mask", [128, 128])  # 4-scene block-diag 0/1
    negdiag = inp("negdiag", [SC, B])         # 1e6 at (i, SC*core + i)

    loss_part = nc.dram_tensor("loss_part", [1, 1], F32, kind="ExternalOutput").ap()

    # ---------------- DRAM scratch ----------------
    ag_in = nc.dram_tensor("ag_in", [AG_SZ], BF16).ap()
    ag_out = nc.dram_tensor("ag_out", [NC_ * AG_SZ], BF16, addr_space="Shared").ap()
    ar_in = nc.dram_tensor("ar_in", [1, 2 * B], F32).ap()
    ar_out = nc.dram_tensor("ar_out", [1, 2 * B], F32, addr_space="Shared").ap()

    with tile.TileContext(nc) as tc, ExitStack() as ctx:
        # ---------------- long-lived pools (bufs is PER TAG) ----------------
        const = ctx.enter_context(tc.tile_pool(name="const", bufs=1))
        xTp = ctx.enter_context(tc.tile_pool(name="xT", bufs=1))
        rowp = ctx.enter_context(tc.tile_pool(name="rows", bufs=3))
        mrow = ctx.enter_context(tc.tile_pool(name="mrow", bufs=1))
        bcp = ctx.enter_context(tc.tile_pool(name="bc", bufs=3))
        mbcp = ctx.enter_context(tc.tile_pool(name="mbc", bufs=1))
        colp = ctx.enter_context(tc.tile_pool(name="colp", bufs=1))
        col2p = ctx.enter_context(tc.tile_pool(name="col2p", bufs=2))
        outp = ctx.enter_context(tc.tile_pool(name="outp", bufs=1))
        mtch = ctx.enter_context(tc.tile_pool(name="mtch", bufs=1))
        m2p = ctx.enter_context(tc.tile_pool(name="m2p", bufs=2))
        wT_p = ctx.enter_context(tc.tile_pool(name="wTp", bufs=8))

        ps_mm = ctx.enter_context(tc.tile_pool(name="ps_mm", bufs=2, space="PSUM"))
        ps_acc = ctx.enter_context(tc.tile_pool(name="ps_acc", bufs=4, space="PSUM"))
        ps_at = ctx.enter_context(tc.tile_pool(name="ps_at", bufs=2, space="PSUM"))

        # ---------------- constants ----------------
        ident = const.tile([128, 128], F32)
        make_identity(nc, ident[:])
        ident_bf = const.tile([128, 128], BF16)
        nc.vector.tensor_copy(ident_bf[:], ident[:])
        ones_col = const.tile([128, 1], F32)
        nc.vector.memset(ones_col[:], 1.0)
        mask_t = const.tile([128, 128], F32)
        nc.sync.dma_start(mask_t[:], attn_mask)
        pn_t = const.tile([SC, B], F32)
        nc.sync.dma_start(pn_t[:], pn_bc)
        nd_t = const.tile([SC, B], F32)
        nc.sync.dma_start(nd_t[:], negdiag)
        eps_col = const.tile([1, 1], F32)
        nc.vector.memset(eps_col[:], LN_EPS)
        m20_col = const.tile([SC, 1], F32)
        nc.vector.memset(m20_col[:], -M_SHIFT)

        # residual stream X_T: KD tiles [128, T] fp32, persistent
        xT = [xTp.tile([128, T], F32, tag=f"xT{k}", name=f"xT{k}") for k in range(KD)]

        def col(ap_1d, base, tag, pool=colp, n=128):
            t = pool.tile([n, 1], F32, tag=tag, name=tag)
            nc.sync.dma_start(t[:], ap_1d[base:base + n])
            return t

        # ============ generator scope (pools released before matching) ======
        with ExitStack() as gctx:
            hp = gctx.enter_context(tc.tile_pool(name="h", bufs=3))
            sqp = gctx.enter_context(tc.tile_pool(name="sq", bufs=4))
            bw_p = gctx.enter_context(tc.tile_pool(name="bigw", bufs=KD))
            wo_p = gctx.enter_context(tc.tile_pool(name="wo", bufs=KD))
            w2_p = gctx.enter_context(tc.tile_pool(name="w2", bufs=16))
            qk_p = gctx.enter_context(tc.tile_pool(name="qk", bufs=12))
            v65_p = gctx.enter_context(tc.tile_pool(name="v65", bufs=3))
            e_p = gctx.enter_context(tc.tile_pool(name="et", bufs=2))
            onat_p = gctx.enter_context(tc.tile_pool(name="onat", bufs=3))
            oT_p = gctx.enter_context(tc.tile_pool(name="oT", bufs=4))
            relu_p = gctx.enter_context(tc.tile_pool(name="relu", bufs=4))

            # ========= input projection: X_T = (eps @ in_w.T).T =========
            inw_sb = bw_p.tile([128, FF], F32, tag="bigw", name="inw")
            nc.sync.dma_start(inw_sb[:, 0:D], inwT)
            ib_cols = [col(inb, k * 128, f"bo{k}") for k in range(KD)]
            for b_ in range(NB):
                bsl = slice(b_ * TB, (b_ + 1) * TB)
                eps_blk = sqp.tile([128, TB], F32, tag="sq", name="epsblk")
                nc.sync.dma_start(eps_blk[:], epsT[:, bsl])
                for dt_ in range(KD):
                    ps = ps_mm.tile([128, TB], F32, tag="mm", name="ps")
                    nc.tensor.matmul(ps[:], inw_sb[:, dt_ * 128:(dt_ + 1) * 128],
                                     eps_blk[:], start=True, stop=True)
                    nc.scalar.activation(xT[dt_][:, bsl], ps[:],
                                         AF.Identity, bias=ib_cols[dt_][:], scale=1.0)

            # ========= per-block layernorm =========
            def ln_block(b_, g_cols, b_cols, htag):
                """LN over partition dim for tokens of block b_ -> h tiles."""
                bsl = slice(b_ * TB, (b_ + 1) * TB)
                s_row = rowp.tile([1, TB], F32, tag="srow", name="srow")
                ps_s = ps_mm.tile([1, TB], F32, tag="mm", name="ps")
                for k in range(KD):
                    nc.tensor.matmul(ps_s[:], ones_col[:], xT[k][:, bsl],
                                     start=(k == 0), stop=(k == KD - 1))
                nc.vector.tensor_scalar_mul(s_row[:], ps_s[:], -1.0 / D)   # -mean
                q_row = rowp.tile([1, TB], F32, tag="qrow", name="qrow")
                ps_q = ps_mm.tile([1, TB], F32, tag="mm", name="ps")
                for k in range(KD):
                    sq = sqp.tile([128, TB], F32, tag="sq", name="sq")
                    nc.vector.tensor_mul(sq[:], xT[k][:, bsl], xT[k][:, bsl])
                    nc.tensor.matmul(ps_q[:], ones_col[:], sq[:],
                                     start=(k == 0), stop=(k == KD - 1))
                msq = rowp.tile([1, TB], F32, tag="msq", name="msq")
                nc.vector.tensor_mul(msq[:], s_row[:], s_row[:])
                # var = q/D - m^2  (into q_row)
                nc.vector.scalar_tensor_tensor(q_row[:], ps_q[:], 1.0 / D, msq[:],
                                               op0=ALU.mult, op1=ALU.subtract)
                # rstd = 1/sqrt(var + eps): sqrt into msq, recip into q_row
                nc.scalar.activation(msq[:], q_row[:], AF.Sqrt, bias=eps_col[:], scale=1.0)
                nc.vector.reciprocal(q_row[:], msq[:])
                # shift = -m * rstd (into s_row)
                nc.vector.tensor_mul(s_row[:], s_row[:], q_row[:])
                rstd_bc = bcp.tile([128, TB], F32, tag="rstd_bc", name="rstdbc")
                nc.gpsimd.partition_broadcast(rstd_bc[:], q_row[:])
                shift_bc = bcp.tile([128, TB], F32, tag="shift_bc", name="shiftbc")
                nc.gpsimd.partition_broadcast(shift_bc[:], s_row[:])
                hs = []
                for k in range(KD):
                    hh = hp.tile([128, TB], F32, tag=f"{htag}{k}", name=f"{htag}{k}")
                    nc.vector.tensor_mul(hh[:], xT[k][:, bsl], rstd_bc[:])
                    nc.vector.tensor_add(hh[:], hh[:], shift_bc[:])
                    nc.scalar.activation(hh[:], hh[:], AF.Identity,
                                         bias=b_cols[k][:], scale=g_cols[k][:])
                    hs.append(hh)
                return hs

            # ========= transformer layers =========
            for li in range(DEPTH):
                # ---- attention phase ----
                g1c = [col(ln1g[li], k * 128, f"lng{k}") for k in range(KD)]
                lb1c = [col(ln1b[li], k * 128, f"lnb{k}") for k in range(KD)]
                wq_sl = []
                for k in range(KD):
                    w = bw_p.tile([128, FF], F32, tag="bigw", name="wqs")
                    nc.sync.dma_start(w[:, 0:3 * D], wqkvT[li, k * 128:(k + 1) * 128, :])
                    wq_sl.append(w)
                wo_sl = []
                for k in range(KD):
                    w = wo_p.tile([128, D], F32, tag="wo", name="wos")
                    nc.sync.dma_start(w[:], woT[li, k * 128:(k + 1) * 128, :])
                    wo_sl.append(w)
                bq_cols = [col(bqkv[li], ot * 128, f"bq{ot}") for ot in range(8)]
                bv_bc = bcp.tile([128, D], F32, tag="bvbc", name="bvbc", bufs=1)
                nc.gpsimd.dma_start(bv_bc[:], bass.AP(
                    tensor=bqkv.tensor, offset=bqkv.offset + li * 3 * D + 2 * D,
                    ap=[[0, 128], [1, D]]))
                bo_cols = [col(bo[li], ot * 128, f"bo{ot}") for ot in range(KD)]

                for b_ in range(NB):
                    tsl = slice(b_ * TB, (b_ + 1) * TB)
                    h = ln_block(b_, g1c, lb1c, "h")
                    # Q,K projections (transposed out)
                    qk = []
                    for ot in range(8):
                        ps = ps_mm.tile([128, TB], F32, tag="mm", name="ps")
                        for k in range(KD):
                            nc.tensor.matmul(ps[:], wq_sl[k][:, ot * 128:(ot + 1) * 128],
                                             h[k][:], start=(k == 0), stop=(k == KD - 1))
                        t = qk_p.tile([128, TB], F32, tag="qk", name="qk")
                        nc.scalar.activation(t[:], ps[:], AF.Identity,
                                             bias=bq_cols[ot][:], scale=1.0)
                        qk.append(t)
                    onats = []
                    for tt in range(NS):
                        ssl = slice(tt * 128, (tt + 1) * 128)
                        # V natural for this subtile, 65-strided with ones column
                        ps = ps_mm.tile([128, D], F32, tag="mm", name="ps")
                        for k in range(KD):
                            nc.tensor.matmul(ps[:], h[k][:, ssl], wq_sl[k][:, 2 * D:3 * D],
                                             start=(k == 0), stop=(k == KD - 1))
                        v = v65_p.tile([128, 8 * 65], F32, tag="v65", name="v65")
                        nc.vector.memset(
                            v[:].rearrange("p (hh c) -> p hh c", hh=8)[:, :, 64:65], 1.0)
                        for hh in range(8):
                            nc.vector.tensor_add(v[:, hh * 65:hh * 65 + 64],
                                                 ps[:, hh * 64:(hh + 1) * 64],
                                                 bv_bc[:, hh * 64:(hh + 1) * 64])
                        # attention
                        onat = onat_p.tile([128, D], F32, tag="onat", name="onat")
                        for hh in range(8):
                            bp = (hh % 2) * 64
                            kt = qk[4 + hh // 2]
                            qt = qk[hh // 2]
                            s_ps = ps_at.tile([128, 128], F32, tag="at", name="sps")
                            nc.tensor.matmul(s_ps[:], kt[bp:bp + 64, ssl], qt[bp:bp + 64, ssl],
                                             start=True, stop=True)
                            et = e_p.tile([128, 128], F32, tag="et", name="et")
                            nc.scalar.activation(et[:], s_ps[:], AF.Exp, bias=0.0, scale=0.125)
                            nc.vector.tensor_mul(et[:], et[:], mask_t[:])
                            o_ps = ps_at.tile([128, 65], F32, tag="at", name="ops")
                            nc.tensor.matmul(o_ps[:], et[:], v[:, hh * 65:(hh + 1) * 65],
                                             start=True, stop=True)
                            rcol = col2p.tile([128, 1], F32, tag="rcol", name="rcol")
                            nc.vector.reciprocal(rcol[:], o_ps[:, 64:65])
                            nc.vector.tensor_scalar_mul(onat[:, hh * 64:(hh + 1) * 64],
                                                        o_ps[:, 0:64], rcol[:])
                        onats.append(onat)
                    # transpose O -> O_T
                    oT = [oT_p.tile([128, TB], F32, tag="oT", name="oT") for _ in range(KD)]
                    for tt in range(NS):
                        for k in range(KD):
                            tp = ps_at.tile([128, 128], F32, tag="at", name="tp")
                            nc.tensor.transpose(tp[:], onats[tt][:, k * 128:(k + 1) * 128],
                                                ident[:])
                            nc.vector.tensor_copy(oT[k][:, tt * 128:(tt + 1) * 128], tp[:])
                    # Wo + residual
                    for ot in range(KD):
                        ps = ps_mm.tile([128, TB], F32, tag="mm", name="ps")
                        for k in range(KD):
                            nc.tensor.matmul(ps[:], wo_sl[k][:, ot * 128:(ot + 1) * 128],
                                             oT[k][:], start=(k == 0), stop=(k == KD - 1))
                        nc.vector.scalar_tensor_tensor(xT[ot][:, tsl], ps[:], bo_cols[ot][:],
                                                       xT[ot][:, tsl], op0=ALU.add, op1=ALU.add)

                # ---- FF phase ----
                g2c = [col(ln2g[li], k * 128, f"lng{k}") for k in range(KD)]
                lb2c = [col(ln2b[li], k * 128, f"lnb{k}") for k in range(KD)]
                w1_sl = []
                for k in range(KD):
                    w = bw_p.tile([128, FF], F32, tag="bigw", name="w1s")
                    nc.sync.dma_start(w[:], w1T[li, k * 128:(k + 1) * 128, :])
                    w1_sl.append(w)
                w2_sl = []
                for kf in range(KF):
                    w = w2_p.tile([128, D], F32, tag="w2", name="w2s")
                    nc.scalar.dma_start(w[:], w2T[li, kf * 128:(kf + 1) * 128, :])
                    w2_sl.append(w)
                bff_cols = [col(b2[li], ot * 128, f"bo{ot}") for ot in range(KD)]
                for b_ in range(NB):
                    tsl = slice(b_ * TB, (b_ + 1) * TB)
                    h2 = ln_block(b_, g2c, lb2c, "h")
                    acc = [ps_acc.tile([128, TB], F32, tag="acc", name="facc")[:]
                           for _ in range(KD)]
                    for kf in range(KF):
                        ps = ps_mm.tile([128, TB], F32, tag="mm", name="ps")
                        for k in range(KD):
                            nc.tensor.matmul(ps[:], w1_sl[k][:, kf * 128:(kf + 1) * 128],
                                             h2[k][:], start=(k == 0), stop=(k == KD - 1))
                        b1col = col(b1[li], kf * 128, "b1c", pool=col2p)
                        rl = relu_p.tile([128, TB], F32, tag="relu", name="rl")
                        nc.scalar.activation(rl[:], ps[:], AF.Relu, bias=b1col[:], scale=1.0)
                        for ot in range(KD):
                            nc.tensor.matmul(acc[ot], w2_sl[kf][:, ot * 128:(ot + 1) * 128],
                                             rl[:], start=(kf == 0), stop=(kf == KF - 1))
                    for ot in range(KD):
                        nc.vector.scalar_tensor_tensor(xT[ot][:, tsl], acc[ot],
                                                       bff_cols[ot][:], xT[ot][:, tsl],
                                                       op0=ALU.add, op1=ALU.add)

        # ============ generator pools released here =========================

        # ---------------- matching-stage pools (reuse generator space) ------
        outw_p = ctx.enter_context(tc.tile_pool(name="outw", bufs=1))
        outp2 = ctx.enter_context(tc.tile_pool(name="outp2", bufs=1))
        pnat_p = ctx.enter_context(tc.tile_pool(name="pnat", bufs=1))
        xfa_p = ctx.enter_context(tc.tile_pool(name="xfa", bufs=1))
        xfTs_p = ctx.enter_context(tc.tile_pool(name="xfTs", bufs=2))
        pts_p = ctx.enter_context(tc.tile_pool(name="pts", bufs=4))

        # fp32 local xf in natural layout [scene, feature] (replaces DRAM trip)
        xfl = outp2.tile([SC, FDIM], F32, tag="xfl", name="xfl")
        # y_T [ch, tok] in bf16 (stationary source for matching matmuls)
        yT_bf = outp2.tile([128, T], BF16, tag="yTbf")

        # ========= output projection =========
        outw_sb = outw_p.tile([128, KD * CH], F32, tag="outw", name="outw")
        for k in range(KD):
            nc.sync.dma_start(outw_sb[:, k * CH:(k + 1) * CH],
                              outwT[k * 128:(k + 1) * 128, :])
        outb_col = col(outb, 0, "outbcol")
        outb_bc = mbcp.tile([128, CH], F32, tag="outbbc", name="outbbc")
        nc.gpsimd.dma_start(outb_bc[:], bass.AP(
            tensor=outb.tensor, offset=outb.offset, ap=[[0, 128], [1, CH]]))

        for b_ in range(NB):
            ps = ps_mm.tile([128, TB], F32, tag="mm", name="ps")
            for k in range(KD):
                nc.tensor.matmul(ps[:], outw_sb[:, k * CH:(k + 1) * CH],
                                 xT[k][:, b_ * TB:(b_ + 1) * TB],
                                 start=(k == 0), stop=(k == KD - 1))
            nc.scalar.activation(yT_bf[:, b_ * TB:(b_ + 1) * TB], ps[:], AF.Identity,
                                 bias=outb_col[:], scale=1.0)

        # y natural -> xfl SBUF fp32; bf16 shards straight into ag_in
        for tt in range(T // 128):
            ps = ps_at.tile([128, CH], F32, tag="at", name="yn_ps")
            for k in range(KD):
                nc.tensor.matmul(ps[:], xT[k][:, tt * 128:(tt + 1) * 128],
                                 outw_sb[:, k * CH:(k + 1) * CH],
                                 start=(k == 0), stop=(k == KD - 1))
            yn = m2p.tile([128, CH], F32, tag="yn", name="yn")
            nc.vector.tensor_add(yn[:], ps[:], outb_bc[:])
            nc.sync.dma_start(
                xfl[tt * 4:(tt + 1) * 4, :].rearrange("p (l c) -> p l c", l=L),
                yn[:])
            ynbf = m2p.tile([128, CH], BF16, tag="ynbf", name="ynbf")
            nc.vector.tensor_copy(ynbf[:], yn[:])
            nc.scalar.dma_start(
                ag_in[AG_XFN + tt * 4 * FDIM: AG_XFN + (tt + 1) * 4 * FDIM]
                .rearrange("(i l c) -> i l c", l=L, c=CH),
                ynbf[:])

        # xn = ||xf_i||^2 via gram diag (bf16 inputs, fp32 accum)
        xfT_st = yT_bf[:].rearrange("c (i l) -> c l i", l=L)   # [128, 32, 64]
        gram = ps_at.tile([SC, SC], F32, tag="at", name="gram")
        for l in range(KFl):
            nc.tensor.matmul(gram[:], xfT_st[:, l, :], xfT_st[:, l, :],
                             start=(l == 0), stop=(l == KFl - 1))
        gd = m2p.tile([SC, SC], F32, tag="gd", name="gd")
        nc.vector.tensor_mul(gd[:], gram[:], ident[0:SC, 0:SC])
        xn_col = colp.tile([SC, 1], F32, tag="xncol", name="xncol")
        nc.vector.reduce_sum(xn_col[:], gd[:], axis=AX.X)
        nc.sync.dma_start(
            ag_in[AG_XN:AG_XN + 2 * SC].rearrange("(i bb) -> i bb", bb=2),
            xn_col[:].bitcast(BF16))
        nc.gpsimd.collective_compute(
            "AllGather", ALU.bypass, replica_groups=[list(range(NC_))],
            ins=[ag_in[:]], outs=[ag_out[:]])

        # preload p tiles (no dependence on AG)
        pnat_t = []
        for jt in range(4):
            t = pnat_p.tile([128, FDIM], BF16, tag=f"pn{jt}", name=f"pn{jt}")
            nc.scalar.dma_start(t[:], pnat[jt * 128:(jt + 1) * 128, :])
            pnat_t.append(t)

        # S_pos (does not need AG): acc over 32 f-chunks
        spos = ps_acc.tile([SC, B], F32, tag="acc", name="spos")
        for l in range(KFl):
            mv = pts_p.tile([128, B], BF16, tag="mv", name="mv")
            nc.sync.dma_start(mv[:], pT[l * 128:(l + 1) * 128, :])
            nc.tensor.matmul(spos[:], xfT_st[:, l, :], mv[:],
                             start=(l == 0), stop=(l == KFl - 1))

        # xn_full row [1, 512] f32 + broadcast
        ago_f32 = ag_out.bitcast(F32)
        xn_row = mrow.tile([1, B], F32, tag="mr", name="xnrow")
        nc.sync.dma_start(
            xn_row[:],
            bass.AP(tensor=ago_f32.tensor, offset=ago_f32.offset + AG_XN // 2,
                    ap=[[1, 1], [AG_SZ // 2, NC_], [1, SC]]))
        xn_bc = mbcp.tile([SC, B], F32, tag="mbc", name="xnbc")
        nc.gpsimd.partition_broadcast(xn_bc[:], xn_row[:])

        # xf_nat_all: 4 scene-tiles [128, 4096] bf16 (8KB lines)
        xfa = []
        for st in range(4):
            t = xfa_p.tile([128, FDIM], BF16, tag=f"xfa{st}", name=f"xfa{st}")
            for half in range(2):
                c = 2 * st + half
                nc.sync.dma_start(
                    t[half * SC:(half + 1) * SC, :],
                    bass.AP(tensor=ag_out.tensor,
                            offset=ag_out.offset + c * AG_SZ + AG_XFN,
                            ap=[[FDIM, SC], [1, FDIM]]))
            xfa.append(t)

        # S_neg: rebuild xf^T_all [128 f, 512 scene] per l-chunk via PE
        # transposes (double-buffered), accumulate immediately
        sneg = ps_acc.tile([SC, B], F32, tag="acc", name="sneg")
        for l in range(KFl):
            xfT_l = xfTs_p.tile([128, B], BF16, tag="xfTs", name="xfTs")
            for st in range(4):
                tp = ps_at.tile([128, 128], BF16, tag="at", name="ttp")
                nc.tensor.transpose(tp[:], xfa[st][:, l * 128:(l + 1) * 128],
                                    ident_bf[:])
                nc.vector.tensor_copy(xfT_l[:, st * 128:(st + 1) * 128], tp[:])
            nc.tensor.matmul(sneg[:], xfT_st[:, l, :], xfT_l[:],
                             start=(l == 0), stop=(l == KFl - 1))

        # distances -> logits -> E (in place)
        dist = mtch.tile([SC, 2 * B], F32, tag="dist")
        nc.vector.scalar_tensor_tensor(dist[:, 0:B], spos[:], -2.0, pn_t[:],
                                       op0=ALU.mult, op1=ALU.add)
        nc.vector.scalar_tensor_tensor(dist[:, B:2 * B], sneg[:], -2.0, xn_bc[:],
                                       op0=ALU.mult, op1=ALU.add)
        nc.vector.tensor_scalar_add(dist[:], dist[:], xn_col[:])
        nc.vector.tensor_scalar_max(dist[:], dist[:], 0.0)
        nc.scalar.activation(dist[:], dist[:], AF.Sqrt, bias=0.0, scale=1.0)
        nc.vector.tensor_add(dist[:, B:2 * B], dist[:, B:2 * B], nd_t[:])
        dmin = colp.tile([SC, 1], F32, tag="dmin", name="dmin")
        nc.vector.tensor_reduce(out=dmin[:], in_=dist[:], axis=AX.X, op=ALU.min)
        E = dist  # in place: E = exp(-d + dmin)
        nc.scalar.activation(E[:], dist[:], AF.Exp, bias=dmin[:], scale=-1.0)
        g_col = colp.tile([SC, 1], F32, tag="gcol", name="gcol")
        nc.scalar.activation(g_col[:], dmin[:], AF.Exp, bias=m20_col[:], scale=-1.0)
        sr_col = colp.tile([SC, 1], F32, tag="srcol", name="srcol")
        nc.vector.reduce_sum(sr_col[:], E[:], axis=AX.X)
        # partial colsums of G = E * g_i via g-weighted stationary
        cs_row = mrow.tile([1, 2 * B], F32, tag="mr", name="csrow")
        for b_ in range(2):
            ps = ps_mm.tile([1, B], F32, tag="mm", name="ps")
            nc.tensor.matmul(ps[:], g_col[:], E[:, b_ * B:(b_ + 1) * B],
                             start=True, stop=True)
            nc.vector.tensor_copy(cs_row[:, b_ * B:(b_ + 1) * B], ps[:])
        nc.sync.dma_start(ar_in, cs_row[:])
        nc.gpsimd.collective_compute(
            "AllReduce", ALU.add, replica_groups=[list(range(NC_))],
            ins=[ar_in[:]], outs=[ar_out[:]])
        cs_g = mrow.tile([1, 2 * B], F32, tag="mr", name="csg")
        nc.sync.dma_start(cs_g[:], ar_out)
        cs_bc = mbcp.tile([SC, 2 * B], F32, tag="csbc", name="csbc")
        nc.gpsimd.partition_broadcast(cs_bc[:], cs_g[:])
        nc.scalar.activation(cs_bc[:], cs_bc[:], AF.Sqrt, bias=0.0, scale=1.0)
        nc.vector.reciprocal(cs_bc[:], cs_bc[:])
        # E' = E * invsqrt(Sc); row scalars BEFORE overwriting E with W
        nc.vector.tensor_mul(E[:], E[:], cs_bc[:])
        snp = colp.tile([SC, 1], F32, tag="snp", name="snp")
        nc.vector.reduce_sum(snp[:], E[:, B:2 * B], axis=AX.X)
        spp = colp.tile([SC, 1], F32, tag="spp", name="spp")
        nc.vector.reduce_sum(spp[:], E[:, 0:B], axis=AX.X)
        tcol = colp.tile([SC, 1], F32, tag="tcol", name="tcol")
        nc.vector.reciprocal(tcol[:], sr_col[:])
        nc.vector.tensor_mul(tcol[:], tcol[:], g_col[:])
        ccol = colp.tile([SC, 1], F32, tag="ccol", name="ccol")
        nc.scalar.activation(ccol[:], tcol[:], AF.Sqrt, bias=0.0, scale=1.0)
        alpha = colp.tile([SC, 1], F32, tag="alpha", name="alpha")
        nc.vector.tensor_mul(alpha[:], tcol[:], snp[:])
        beta = colp.tile([SC, 1], F32, tag="beta", name="beta")
        nc.vector.tensor_mul(beta[:], alpha[:], spp[:])
        nc.vector.tensor_mul(beta[:], beta[:], ccol[:])
        nc.vector.tensor_scalar_mul(beta[:], beta[:], -1.0)
        # W = E' * alpha / -beta (in place), transpose, cast bf16
        nc.vector.tensor_scalar_mul(E[:, 0:B], E[:, 0:B], alpha[:])
        nc.vector.tensor_scalar_mul(E[:, B:2 * B], E[:, B:2 * B], beta[:])
        wT = []
        for half in range(2):
            for jt in range(4):
                tp = ps_at.tile([128, SC], F32, tag="at", name="wtp")
                nc.tensor.transpose(
                    tp[:], E[:, half * B + jt * 128: half * B + (jt + 1) * 128],
                    ident[0:SC, 0:SC])
                t = wT_p.tile([128, SC], BF16, tag="wT", name="wT")
                nc.vector.tensor_copy(t[:], tp[:])
                wT.append(t)
        # V and loss: V = Wpos @ p - Wneg @ xf_full, r = xf - fl(xf + V)
        # everything SBUF-resident
        lacc = m2p.tile([SC, 16], F32, tag="lacc", name="lacc", bufs=1)
        FBW = 256
        for fb in range(FDIM // FBW):
            fsl = slice(fb * FBW, (fb + 1) * FBW)
            vps = ps_acc.tile([SC, FBW], F32, tag="acc", name="vps")
            for jt in range(4):
                nc.tensor.matmul(vps[:], wT[jt][:], pnat_t[jt][:, fsl],
                                 start=(jt == 0), stop=False)
            for jt in range(4):
                nc.tensor.matmul(vps[:], wT[4 + jt][:], xfa[jt][:, fsl],
                                 start=False, stop=(jt == 3))
            t1 = m2p.tile([SC, FBW], F32, tag="t1", name="t1")
            nc.vector.tensor_add(t1[:], xfl[:, fsl], vps[:])
            nc.vector.tensor_sub(t1[:], xfl[:, fsl], t1[:])
            nc.vector.tensor_mul(t1[:], t1[:], t1[:])
            nc.vector.reduce_sum(lacc[:, fb:fb + 1], t1[:], axis=AX.X)
        lsum = colp.tile([SC, 1], F32, tag="lsum", name="lsum")
        nc.vector.reduce_sum(lsum[:], lacc[:], axis=AX.X)
        tot = ps_mm.tile([1, 1], F32, tag="mm", name="tot")
        nc.tensor.matmul(tot[:], ones_col[0:SC, :], lsum[:], start=True, stop=True)
        tot_sb = colp.tile([1, 1], F32, tag="tot", name="totsb")
        nc.vector.tensor_copy(tot_sb[:], tot[:])
        nc.sync.dma_start(loss_part, tot_sb[:])

    nc.compile()
    return nc


_NC_CACHE = None


def _get_nc():
    global _NC_CACHE
    if _NC_CACHE is None:
        _NC_CACHE = _build_nc()
    return _NC_CACHE


def _prep_inputs(inputs):
    f32 = lambda x: np.ascontiguousarray(np.asarray(x), dtype=np.float32)
    bf = lambda x: np.ascontiguousarray(np.asarray(x, dtype=ml_dtypes.bfloat16))
    sample_p = f32(inputs["sample_p"])
    eps = f32(inputs["eps"])
    p2 = sample_p.reshape(B, FDIM)
    pn = (p2.astype(np.float64) ** 2).sum(-1).astype(np.float32)

    common = {
        "inwT": f32(inputs["in_w"]).T.copy(),
        "inb": f32(inputs["in_b"]),
        "wqkvT": np.ascontiguousarray(f32(inputs["Wqkv"]).transpose(0, 2, 1)),
        "bqkv": f32(inputs["bqkv"]),
        "woT": np.ascontiguousarray(f32(inputs["Wo"]).transpose(0, 2, 1)),
        "bo": f32(inputs["bo"]),
        "ln1g": f32(inputs["ln1_g"]), "ln1b": f32(inputs["ln1_b"]),
        "w1T": np.ascontiguousarray(f32(inputs["W1"]).transpose(0, 2, 1)),
        "b1": f32(inputs["b1"]),
        "w2T": np.ascontiguousarray(f32(inputs["W2"]).transpose(0, 2, 1)),
        "b2": f32(inputs["b2"]),
        "ln2g": f32(inputs["ln2_g"]), "ln2b": f32(inputs["ln2_b"]),
        "outwT": f32(inputs["out_w"]).T.copy(),
        "outb": f32(inputs["out_b"]),
        "pT": bf(p2.T),
        "pnat": bf(p2),
        "pn_bc": np.broadcast_to(pn[None, :], (SC, B)).copy(),
        "attn_mask": np.kron(np.eye(4, dtype=np.float32), np.ones((32, 32), np.float32)),
    }
    in_maps = []
    for c in range(NC_):
        nd = np.zeros((SC, B), np.float32)
        nd[np.arange(SC), SC * c + np.arange(SC)] = 1e6
        m = dict(common)
        m["epsT"] = eps[c * SC:(c + 1) * SC].reshape(T, CH).T.copy()
        m["negdiag"] = nd
        in_maps.append(m)
    return in_maps


def kernel(**inputs) -> np.ndarray:
    nc = _get_nc()
    in_maps = _prep_inputs(inputs)
    res = run_bass_kernel_spmd(nc, in_maps, list(range(NC_)))
    total = sum(float(r["loss_part"][0, 0]) for r in res.results)
    return np.float32(total / (B * FDIM))
